# revision 26
# baseline (speedup 1.0000x reference)
"""Trainium2 Bass kernel for nn_Net_65335042507316.

Equilibrium-propagation-style net: layers 784 -> 500 -> 500 -> 500 -> 10.
500 free + 8 clamped Jacobi fixed-point iterations over unit states, B=256.

Since all states start at 0 and every update passes through clip(0,1), the
d_rho masks are identically 1, so one iteration is (eps=0.5, beta=1):

    n1 = clip(0.5*u1 + 0.5*(x@W0+b0) + 0.5*(u2@W1.T))
    n2 = clip(0.5*u2 + 0.5*(u1@W1 + u3@W2.T + b1))
    n3 = clip(0.5*u3 + 0.5*(u2@W2 + u4@W3.T + b2))
    n4 = clip(0.5*u4 + 0.5*(u3@W3 + b3))            (free)
    n4 = clip(0.5*(u3@W3 + b3) + 0.5*t)             (clamped)

Sharding: data-parallel over batch, 32 samples per core, weights replicated,
no cross-core communication.  The dynamics are chaotic (bf16 or reduced-
precision weights give errors far outside the fp32 envelope) so all matmuls
are fp32.

Layout: states are feature-major, folded as SBUF tiles [128, 4*32] where
column block mc holds features [mc*128, (mc+1)*128) for the 32 local samples.
Feature index 500 is a constant-one row used to fold the biases into the
weight matrices (which also carry the 0.5 prefactor).  Matmuls are
weight-stationary: lhsT = weight block [128, 128], rhs = state block [128, 32].

The two tiny u4 products are folded into the big matmuls:
  - u4 lives in s2's padding features 501..510 (partitions 117..126 of the
    last chunk), so u4@W3.T rides along u2's D-matmul (extra lhsT rows);
  - u3@W3 (+b3) rides along u3's C-matmul as extra lhsT columns 501..510,
    so n4-pre appears in p2's padding columns.

This gives 64 fp32 matmuls [128x128]@[128x32] per iteration on the PE, with
the DVE doing ~11 update instructions, overlapped.

All constant inputs are packed into two blob tensors (one DMA each) and all
outputs into one blob.  A post-scheduling legalization pass hoists excess
per-instruction sync waits into single-wait NoOps (this walrus build rejects
any instruction with more than ONE sync-wait command).
"""

import numpy as np

import concourse.bass as bass
import concourse.mybir as mybir
import concourse.tile as tile
from concourse.bass_utils import run_bass_kernel_spmd
from concourse.tile_rust import add_dep_helper

F32 = mybir.dt.float32
ADD = mybir.AluOpType.add
MULT = mybir.AluOpType.mult
MIN = mybir.AluOpType.min
MAX = mybir.AluOpType.max

N_CORES = 8
BPC = 32          # batch per core
FP = 512          # padded feature dim for u1..u3 (ones-feature at 500)
NK = 4            # K chunks of 128
XK = 7            # K chunks for padded x dim (896; ones-feature at 784)
NB = FP // 128    # 4
U4P = 117         # u4 features live at partitions 117..126 (= features
                  # 501..510 of the last chunk) so every access is DVE-legal

N_ITER_FREE = 500
N_ITER_CLAMPED = 8

# column offsets inside wblob
_OFF_AW = [k * FP for k in range(NK)]
_OFF_BW = [(NK + k) * FP for k in range(NK)]
_OFF_CW = [(2 * NK + k) * FP for k in range(NK)]
_OFF_DW = [(3 * NK + k) * FP for k in range(NK)]
_OFF_W0 = [(4 * NK + k) * FP for k in range(XK)]
_OFF_TSEL = _OFF_W0[-1] + FP               # 11776: clamped-phase th injector
_OFF_MUSEL = _OFF_TSEL + 128               # clamped-phase -0.5*u4 canceller
W_COLS = _OFF_MUSEL + 128                  # 12032

# column offsets inside dblob
_OFF_XT = 0
_OFF_TH = XK * BPC                         # 224
_OFF_S = [_OFF_TH + BPC + i * NB * BPC for i in range(3)]  # 256,384,512
D_COLS = _OFF_S[2] + NB * BPC              # 640

# column offsets inside oblob (free phase at 0, clamped at _OCL)
_OST = [0, NB * BPC, 2 * NB * BPC]                     # 0,128,256
_OCL = 3 * NB * BPC                                    # 384
O_COLS = 2 * _OCL                                      # 768


# Empirical walrus codegen limit: at most ONE sync-wait command per
# instruction ("Too many sync wait commands" otherwise).
_WAIT_LIMITS = {}
_WAIT_LIMIT_DEFAULT = 1


def _legalize_sync_waits(nc):
    """Hoist excess sync waits into same-engine NoOps inserted just before
    the offending instruction (the engine executes its stream in order, so
    a preceding NoOp carrying part of the wait set is equivalent)."""
    import bass_rust
    counter = [0]
    for func in nc.m.functions:
        for bb in func.blocks:
            insts = bb.instructions
            out = []
            changed = False
            for ins in insts:
                si = ins.sync_info
                waits = list(si.on_wait) if si is not None else []
                lim = _WAIT_LIMITS.get(type(ins).__name__, _WAIT_LIMIT_DEFAULT)
                if len(waits) > lim:
                    spill = waits[lim:]
                    while spill:
                        chunk, spill = spill[:1], spill[1:]
                        counter[0] += 1
                        nop = bass_rust.InstNoOp(
                            name=f"lsw_nop_{counter[0]}", engine=ins.engine,
                            ins=[], outs=[])
                        nop.sync_info = bass_rust.SyncInfo(
                            on_wait=chunk, on_update=[])
                        out.append(nop)
                    si.on_wait = waits[:lim]
                    changed = True
                out.append(ins)
            if changed:
                bb.instructions = out
    return nc


def build_nc(n_free=N_ITER_FREE, n_clamped=N_ITER_CLAMPED, unroll=4,
             legalize=True):
    nc = bass.Bass()

    wblob_d = nc.dram_tensor("wblob", [128, W_COLS], F32, kind="ExternalInput")
    dblob_d = nc.dram_tensor("dblob", [128, D_COLS], F32, kind="ExternalInput")
    oblob_d = nc.dram_tensor("oblob", [128, O_COLS], F32, kind="ExternalOutput")

    with tile.TileContext(nc) as tc:
        with (
            tc.tile_pool(name="weights", bufs=1) as wpool,
            tc.tile_pool(name="state", bufs=1) as spool,
            tc.tile_pool(name="tmp", bufs=4) as tpool,
            tc.tile_pool(name="psum", bufs=2, space="PSUM") as ppool,
        ):
            # All input data is funneled through DVE copies so that every
            # downstream instruction (in particular fp32 self-loading
            # matmuls) only ever waits on the DVE semaphore.  The DMAs live
            # in a one-trip loop whose exit barrier ratifies the DMA-queue
            # ticks within the SP sequencer's wait budget.
            wt_dma = wpool.tile([128, W_COLS], F32, tag="wt_dma")
            db_dma = wpool.tile([128, D_COLS], F32, tag="db_dma")
            with tc.For_i(0, 1, 1):
                dma1 = nc.sync.dma_start(wt_dma, wblob_d[:, :])
                dma2 = nc.sync.dma_start(db_dma, dblob_d[:, :])

            wt = wpool.tile([128, W_COLS], F32, tag="wt")
            s1 = spool.tile([128, NB * BPC], F32, tag="s1")
            s2 = spool.tile([128, NB * BPC], F32, tag="s2")
            s3 = spool.tile([128, NB * BPC], F32, tag="s3")
            xtile = spool.tile([128, XK * BPC], F32, tag="xtile")
            thtile = spool.tile([128, BPC], F32, tag="thtile")
            xc1 = spool.tile([128, NB * BPC], F32, tag="xc1")
            ost = spool.tile([128, _OCL], F32, tag="ost")

            nc.vector.tensor_copy(wt, wt_dma)
            nc.vector.tensor_copy(s1, db_dma[:, _OFF_S[0]:_OFF_S[0] + NB * BPC])
            nc.vector.tensor_copy(s2, db_dma[:, _OFF_S[1]:_OFF_S[1] + NB * BPC])
            nc.vector.tensor_copy(s3, db_dma[:, _OFF_S[2]:_OFF_S[2] + NB * BPC])
            nc.vector.tensor_copy(xtile, db_dma[:, _OFF_XT:_OFF_XT + XK * BPC])
            nc.vector.tensor_copy(thtile, db_dma[:, _OFF_TH:_OFF_TH + BPC])
            # Retire the DMA-staging tiles with DVE writes so nothing later
            # re-waits the DMA queue semaphores.
            nc.vector.memset(wt_dma, 0.0)
            nc.vector.memset(db_dma, 0.0)
            # Advance every engine's observed DMA-queue clock.
            for _eng in (nc.tensor, nc.vector, nc.scalar, nc.gpsimd, nc.sync):
                _n = _eng.nop()
                add_dep_helper(_n.ins, dma1.ins, True, "cover dma clock")
                add_dep_helper(_n.ins, dma2.ins, True, "cover dma clock")

            def mcs(mc):  # batch-column block of psum/state for chunk mc
                return slice(mc * BPC, (mc + 1) * BPC)

            def wsl(off, mc):  # weight lhsT block: blob offset off, chunk mc
                return wt[:, off + mc * 128: off + (mc + 1) * 128]

            # ---- xc1 = 0.5*(x@W0 + b0) once ----------------------------
            px = ppool.tile([128, NB * BPC], F32, tag="p1")
            for mc in range(NB):
                for kc in range(XK):
                    nc.tensor.matmul(
                        px[:, mcs(mc)], wsl(_OFF_W0[kc], mc),
                        xtile[:, kc * BPC:(kc + 1) * BPC],
                        start=(mc == 0 and kc == 0),
                        stop=(mc == NB - 1 and kc == XK - 1))
            nc.vector.tensor_copy(xc1, px)

            # ---- one Jacobi iteration ----------------------------------
            # `prev` = (last_dve_inst, last_mm_inst) of the previous
            # iteration in the same basic block, or None right after a drain
            # barrier.  A PE nop at the head of the iteration absorbs the
            # DVE wait so no fp32 self-loading matmul carries more than one
            # sync wait; ONE start=True per psum bank per iteration avoids
            # PE completion-waits (the bank-wide has_written clear makes
            # later first-touches overwrite correctly).
            def emit_iter(clamped, prev=None):
                pe_nop = None
                if prev is not None:
                    pe_nop = nc.tensor.nop()
                    for dep in prev:
                        add_dep_helper(pe_nop.ins, dep.ins, True,
                                       "iter wait absorber")

                p1 = ppool.tile([128, NB * BPC], F32, tag="p1")
                p2 = ppool.tile([128, NB * BPC], F32, tag="p2")
                p3 = ppool.tile([128, NB * BPC], F32, tag="p3")

                # B: p2 += 0.5*W1 (stream s1); C: p2 += 0.5*W2.T (stream s3)
                # C also carries the folded 0.5*W3 + b3 -> n4-pre in p2's
                # padding columns 501..510.
                for mc in range(NB):
                    for kc in range(NK):
                        mm = nc.tensor.matmul(p2[:, mcs(mc)],
                                              wsl(_OFF_BW[kc], mc),
                                              s1[:, mcs(kc)],
                                              start=(mc == 0 and kc == 0),
                                              stop=False)
                        if pe_nop is not None and mc == 0 and kc == 0:
                            add_dep_helper(mm.ins, pe_nop.ins, False,
                                           "keep absorber first")
                    if clamped and mc == 0:
                        # inject 0.5*t and cancel the 0.5*u4 self-term that
                        # the shared s2-update adds (clamped n4 has neither)
                        nc.tensor.matmul(p2[:, mcs(NB - 1)],
                                         wt[:, _OFF_TSEL:_OFF_TSEL + 128],
                                         thtile[:, :], start=False, stop=False)
                        nc.tensor.matmul(p2[:, mcs(NB - 1)],
                                         wt[:, _OFF_MUSEL:_OFF_MUSEL + 128],
                                         s2[:, mcs(NB - 1)], start=False,
                                         stop=False)
                    for kc in range(NK):
                        nc.tensor.matmul(p2[:, mcs(mc)], wsl(_OFF_CW[kc], mc),
                                         s3[:, mcs(kc)], start=False,
                                         stop=(mc == NB - 1 and kc == NK - 1))
                # A: p1 = 0.5*W1.T (stream s2)
                for mc in range(NB):
                    for kc in range(NK):
                        nc.tensor.matmul(p1[:, mcs(mc)], wsl(_OFF_AW[kc], mc),
                                         s2[:, mcs(kc)],
                                         start=(mc == 0 and kc == 0),
                                         stop=(mc == NB - 1 and kc == NK - 1))
                # D: p3 = 0.5*W2 (stream s2; lhsT rows 501..510 carry the
                # folded 0.5*W3.T acting on u4, which lives in s2's padding)
                last_mm = None
                for mc in range(NB):
                    for kc in range(NK):
                        last_mm = nc.tensor.matmul(
                            p3[:, mcs(mc)], wsl(_OFF_DW[kc], mc),
                            s2[:, mcs(kc)], start=(mc == 0 and kc == 0),
                            stop=(mc == NB - 1 and kc == NK - 1))

                # updates; the s2 update's pad rows ARE the u4 update
                # (0.5*u4 + n4pre, then clip), so u4 needs no extra ops.
                t1 = tpool.tile([128, NB * BPC], F32, tag="t1")
                nc.vector.scalar_tensor_tensor(t1, s1, 0.5, p1, MULT, ADD)
                nc.vector.tensor_tensor(t1, t1, xc1, ADD)
                c1 = nc.vector.tensor_scalar(s1, t1, 1.0, 0.0, MIN, MAX)

                t2 = tpool.tile([128, NB * BPC], F32, tag="t2")
                nc.vector.scalar_tensor_tensor(t2, s2, 0.5, p2, MULT, ADD)
                c2 = nc.vector.tensor_scalar(s2, t2, 1.0, 0.0, MIN, MAX)

                t3 = tpool.tile([128, NB * BPC], F32, tag="t3")
                nc.vector.scalar_tensor_tensor(t3, s3, 0.5, p3, MULT, ADD)
                c3 = nc.vector.tensor_scalar(s3, t3, 1.0, 0.0, MIN, MAX)
                return (c1, c2, c3, last_mm)

            # ---- free phase --------------------------------------------
            n_loop, n_rem = divmod(n_free, unroll)
            if n_loop > 0:
                with tc.For_i(0, n_loop, 1,
                              hint_engines=(mybir.EngineType.PE,)) as _i:
                    prev = None
                    for _ in range(unroll):
                        prev = emit_iter(False, prev)
            prev = None
            for _ in range(n_rem):
                prev = emit_iter(False, prev)

            for i, s in enumerate((s1, s2, s3)):
                nc.scalar.copy(ost[:, _OST[i]:_OST[i] + NB * BPC], s)
            nc.sync.dma_start(oblob_d[:, 0:_OCL], ost)

            # ---- clamped phase -----------------------------------------
            for _ in range(n_clamped):
                prev = emit_iter(True, prev)

            ost2 = spool.tile([128, _OCL], F32, tag="ost2")
            for i, s in enumerate((s1, s2, s3)):
                nc.scalar.copy(ost2[:, _OST[i]:_OST[i] + NB * BPC], s)
            nc.sync.dma_start(oblob_d[:, _OCL:O_COLS], ost2)

    return _legalize_sync_waits(nc) if legalize else nc


# ---------------------------------------------------------------------------
# Host-side data prep
# ---------------------------------------------------------------------------

def _prep_shared(W0, W1, W2, W3, b0, b1, b2, b3):
    f32 = np.float32
    wb = np.zeros((128, W_COLS), f32)

    def put(off, arr):
        K, C = arr.shape
        for k in range((K + 127) // 128):
            lo, hi = k * 128, min((k + 1) * 128, K)
            wb[:hi - lo, off[k]:off[k] + C] = arr[lo:hi]

    aw = np.zeros((FP, FP), f32)
    aw[:500, :500] = 0.5 * W1.T
    bw = np.zeros((FP, FP), f32)
    bw[:500, :500] = 0.5 * W1
    bw[500, :500] = 0.5 * b1
    bw[500, 500] = 0.5
    cw = np.zeros((FP, FP), f32)
    cw[:500, :500] = 0.5 * W2.T
    cw[:500, 501:511] = 0.5 * W3              # folded F: n4-pre in p2 pad cols
    cw[500, 501:511] = 0.5 * b3
    dw = np.zeros((FP, FP), f32)
    dw[:500, :500] = 0.5 * W2
    dw[500, :500] = 0.5 * b2
    dw[500, 500] = 0.5
    dw[501:511, :500] = 0.5 * W3.T            # folded E: u4 rides in s2 pad
    w0 = np.zeros((XK * 128, FP), f32)
    w0[:784, :500] = 0.5 * W0
    w0[784, :500] = 0.5 * b0
    w0[784, 500] = 0.5

    put(_OFF_AW, aw)
    put(_OFF_BW, bw)
    put(_OFF_CW, cw)
    put(_OFF_DW, dw)
    put(_OFF_W0, w0)
    for j in range(10):
        wb[U4P + j, _OFF_TSEL + U4P + j] = 1.0     # th injector
        wb[U4P + j, _OFF_MUSEL + U4P + j] = -0.5   # u4 self-term canceller
    return wb


def _fold_state(u_c, nb):
    """[32, nfeat] batch-major -> folded [128, nb*32] with ones-feature."""
    f32 = np.float32
    nfeat = u_c.shape[1]
    s3d = np.zeros((128, nb, BPC), f32)
    uT = u_c.T.astype(f32)
    for mc in range(nb):
        lo = mc * 128
        hi = min(lo + 128, nfeat)
        if hi > lo:
            s3d[:hi - lo, mc, :] = uT[lo:hi, :]
    s3d[nfeat - (nfeat // 128) * 128, nfeat // 128, :] = 1.0
    return s3d.reshape(128, nb * BPC)


def _prep_core(x_c, t_c, u1_c, u2_c, u3_c, u4_c):
    f32 = np.float32
    db = np.zeros((128, D_COLS), f32)
    xt3 = np.zeros((128, XK, BPC), f32)
    xT = x_c.T.astype(f32)                   # [784, 32]
    for kc in range(XK):
        lo = kc * 128
        hi = min(lo + 128, 784)
        if hi > lo:
            xt3[:hi - lo, kc, :] = xT[lo:hi, :]
    xt3[784 - 6 * 128, 6, :] = 1.0           # ones-feature 784
    db[:, _OFF_XT:_OFF_XT + XK * BPC] = xt3.reshape(128, XK * BPC)
    db[U4P:U4P + 10, _OFF_TH:_OFF_TH + BPC] = 0.5 * t_c.T
    s2f = _fold_state(u2_c, NB)
    s2f[U4P:U4P + 10, 3 * BPC:4 * BPC] = u4_c.T.astype(f32)  # u4 in s2 pad
    db[:, _OFF_S[0]:_OFF_S[0] + NB * BPC] = _fold_state(u1_c, NB)
    db[:, _OFF_S[1]:_OFF_S[1] + NB * BPC] = s2f
    db[:, _OFF_S[2]:_OFF_S[2] + NB * BPC] = _fold_state(u3_c, NB)
    return db


def _decode(arr, nfeat):
    # arr [128, nb*32] -> [32, nfeat]
    nb = arr.shape[1] // BPC
    a3 = arr.reshape(128, nb, BPC)
    return np.ascontiguousarray(
        a3.transpose(2, 1, 0).reshape(BPC, nb * 128)[:, :nfeat])


LAST_RESULTS = None


def kernel(x, u1, u2, u3, u4, t, W0, W1, W2, W3, b0, b1, b2, b3,
           n_free=N_ITER_FREE, n_clamped=N_ITER_CLAMPED, unroll=4, _nc=None,
           _trace=False):
    global LAST_RESULTS
    wb = _prep_shared(np.asarray(W0), np.asarray(W1), np.asarray(W2),
                      np.asarray(W3), np.asarray(b0), np.asarray(b1),
                      np.asarray(b2), np.asarray(b3))
    x = np.asarray(x)
    t = np.asarray(t)
    u1, u2, u3, u4 = (np.asarray(u) for u in (u1, u2, u3, u4))
    in_maps = []
    for c in range(N_CORES):
        sl = slice(c * BPC, (c + 1) * BPC)
        in_maps.append(dict(
            wblob=wb,
            dblob=_prep_core(x[sl], t[sl], u1[sl], u2[sl], u3[sl], u4[sl])))

    nc = _nc if _nc is not None else build_nc(n_free, n_clamped, unroll)
    LAST_RESULTS = run_bass_kernel_spmd(nc, in_maps, list(range(N_CORES)),
                                        trace=_trace)
    res = LAST_RESULTS.results

    outs = []
    for phase in range(2):
        for i in range(3):
            off = phase * _OCL + _OST[i]
            full = np.concatenate(
                [_decode(res[c]["oblob"][:, off:off + NB * BPC], 500)
                 for c in range(N_CORES)], axis=0)
            outs.append(full.astype(np.float32))
        off = phase * _OCL + _OST[1] + 3 * BPC      # u4 = s2 block pad rows
        u4full = np.concatenate(
            [np.ascontiguousarray(
                res[c]["oblob"][U4P:U4P + 10, off:off + BPC].T)
             for c in range(N_CORES)], axis=0)
        outs.insert(len(outs), u4full.astype(np.float32))
    return tuple(outs)
